# revision 29
# baseline (speedup 1.0000x reference)
"""Graphormer multi-head attention on 8 Trainium2 NeuronCores.

Problem (hardcoded): B=16, N=512, HIDDEN=768, 12 heads x 64.
  q = x @ Wq.T ; k = x @ Wk.T ; v = x @ Wv.T + bv
  scores = (q.k / sqrt(768)) + (spatial + edge)[:, None]
  out = softmax(scores) @ v ; y = out @ Wo.T + bo
Sharding: data-parallel over batch, 2 batches per core on 8 cores.

Per-core kernel strategy:
  - All layout work happens on the HOST: x/spatial/edge are pre-transposed
    and packed; weights are pre-transposed and cast to bf16.  bv is
    folded into bo' = bo + Wo@bv on the host (valid because softmax rows
    sum to 1).  No PE transposes.
  - All matmuls bf16 with fp32 PSUM accumulation; the 1/sqrt(768)
    softmax scale is folded into Wq on the host.
  - Attention in the S^T layout: S.T[nk, nq] = kT.T @ qT per head.
    exp(S^T * s) on ScalarE -> bf16, then one DVE multiply with
    E^T = exp((spatial+edge)^T) shared across heads.
  - PV per head: lhsT = [v_head | ones] (ones live in dedicated slots of
    the v tile), so one matmul chain yields both O^T (64 rows) and the
    softmax denominator replicated across the other 64 rows -- no
    separate row-sum matmuls.  A tiny stride-0-partition DMA broadcasts
    the denominator row into SBUF on the partitions where O^T lives,
    keeping the reciprocal+normalize DVE ops partition-aligned.
  - y = O @ Wo.T + bo' with lhsT = O^T; bo' enters as a K=1 matmul and
    the result is DMA'd to DRAM straight out of PSUM.
  - Emission interleaves batch b+1's projections into batch b's
    attention loop so the PE never starves.
"""

import numpy as np
import ml_dtypes

B, N, H = 16, 512, 768
NH, HD = 12, 64
NCORES = 8
PB = B // NCORES  # batches per core
P = 128
KC = H // P   # 6 hidden chunks of 128
NQC = N // P  # 4 sequence chunks of 128
SCALE = float(H) ** -0.5        # folded into Wq on the host

BFNP = ml_dtypes.bfloat16

_COMPILED = None

# debug toggles (set before _build for HW-vs-sim bisection)
DBG_NO_PBCAST = False    # replace partition_broadcast with memset(1.0)
DBG_NO_ACCUM = False     # replace DMA-accum bias sum with DVE add
DBG_NO_GMEMSET = False   # vsb ones via DVE memset instead of gpsimd
DBG_SEQUENTIAL = False   # no cross-batch interleaving in emission order
DBG_DUMP = False         # DMA b1 intermediates to DRAM debug outputs


def _build():
    import concourse.bacc as bacc
    import concourse.tile as tile
    import concourse.mybir as mybir

    f32 = mybir.dt.float32
    bf16 = mybir.dt.bfloat16
    Exp = mybir.ActivationFunctionType.Exp
    ADD = mybir.AluOpType.add

    nc = bacc.Bacc("TRN2", target_bir_lowering=False, debug=False,
                   enable_asserts=False, num_devices=NCORES)

    xb_d = nc.dram_tensor("xb", [PB, P, KC, N], bf16, kind="ExternalInput").ap()
    sp_d = nc.dram_tensor("spT", [PB, P, NQC, N], f32, kind="ExternalInput").ap()
    ed_d = nc.dram_tensor("edT", [PB, P, NQC, N], f32, kind="ExternalInput").ap()
    wq_d = nc.dram_tensor("wqT", [P, KC, H], bf16, kind="ExternalInput").ap()
    wk_d = nc.dram_tensor("wkT", [P, KC, H], bf16, kind="ExternalInput").ap()
    wv_d = nc.dram_tensor("wvT", [P, KC, H], bf16, kind="ExternalInput").ap()
    wo_d = nc.dram_tensor("woT", [P, KC, H], bf16, kind="ExternalInput").ap()
    bo_d = nc.dram_tensor("bo2", [H], bf16, kind="ExternalInput").ap()
    y_d = nc.dram_tensor("y", [PB, N, H], f32, kind="ExternalOutput").ap()

    with tile.TileContext(nc) as tc:
        with (
            tc.tile_pool(name="consts", bufs=1) as consts,
            tc.tile_pool(name="weights", bufs=1) as weights,
            tc.tile_pool(name="io", bufs=2) as io,
            tc.tile_pool(name="biasp", bufs=2) as biasp,
            tc.tile_pool(name="qk", bufs=2) as qkp,
            tc.tile_pool(name="vp", bufs=1) as vp,
            tc.tile_pool(name="soft", bufs=5) as soft,
            tc.tile_pool(name="zp", bufs=2) as zp,
            tc.tile_pool(name="op", bufs=2) as op_,
            tc.tile_pool(name="psum", bufs=2, space="PSUM") as psum,
        ):
            ones_r1 = consts.tile([1, P], bf16)
            nc.vector.memset(ones_r1, 1.0)

            # v tiles: each head owns a contiguous 128-wide slot holding
            # [ones | v_h]; the ones columns make the PV matmul emit the
            # softmax denominator, replicated across PSUM partitions 0:64,
            # for free (O lands on 64:128).
            vsb = [vp.tile([P, NQC, NH, P], bf16, name=f"vsb{i}")
                   for i in range(PB)]

            # DMA issue order is the PE warm-up critical path: the q
            # projection needs wqT + xb(0) first, then the bias sum (for the
            # first attention exp), then wkT; everything else can trail.
            wqT = weights.tile([P, KC, H], bf16)
            xbt = []
            xb_t0 = io.tile([P, KC, N], bf16, tag="xb", name="xb0")
            for kc in range(KC):
                nc.sync.dma_start(out=wqT[:, kc, :], in_=wq_d[:, kc, :])
                nc.sync.dma_start(out=xb_t0[:, kc, :], in_=xb_d[0, :, kc, :])
            xbt.append(xb_t0)

            def emit_bias_dma(b):
                bsum = biasp.tile([P, NQC, N], f32, tag="bsum", name="bsum")
                esb = biasp.tile([P, NQC, N], f32, tag="esb", name="esb")
                nc.sync.dma_start(out=bsum, in_=sp_d[b])
                nc.sync.dma_start(out=esb, in_=ed_d[b])
                return bsum, esb

            def emit_bias_exp(bs, engine):
                bsum, esb = bs
                engine.tensor_add(bsum, bsum, esb)
                ET = biasp.tile([P, NQC, N], bf16, tag="ET", name="ET")
                nc.scalar.activation(ET, bsum, Exp)
                return ET.rearrange("p c n -> p (c n)")

            qT, kT, OT = {}, {}, {}

            def emit_projqk(b, mi, which="qk"):
                pairs = {"q": ((wqT, qT),), "k": ((wkT, kT),),
                         "qk": ((wqT, qT), (wkT, kT))}[which]
                for wt, dst in pairs:
                    pp = psum.tile([P, N], f32, tag="pp", name="pp")
                    for kc in range(KC):
                        nc.tensor.matmul(
                            pp, wt[:, kc, mi * P:(mi + 1) * P],
                            xbt[b][:, kc, :],
                            start=(kc == 0), stop=(kc == KC - 1))
                    nc.scalar.copy(dst[b][:, mi, :], pp)

            def emit_projv(b, ni):
                for half in range(2):
                    cols = slice(0, N) if half == 0 else slice(N, H)
                    width = cols.stop - cols.start
                    nh = width // HD       # heads covered by this psum tile
                    h0 = half * 8          # first head
                    pv = psum.tile([P, N], f32, tag="pp", name="pv")
                    for kc in range(KC):
                        nc.tensor.matmul(
                            pv[:, 0:width],
                            xbt[b][:, kc, ni * P:(ni + 1) * P],
                            wvT[:, kc, cols],
                            start=(kc == 0), stop=(kc == KC - 1))
                    pvh = pv[:, 0:width].rearrange("p (h d) -> p h d", d=HD)
                    nc.vector.tensor_copy(
                        vsb[b][:, ni, h0:h0 + nh, HD:P], pvh)

            es_t = {}

            def emit_qk_head(b, h, ETflat, mul_engine=None):
                mi, r0 = h // 2, (h % 2) * HD
                rs = slice(r0, r0 + HD)
                es = soft.tile([P, 2 * N * 2], bf16, tag="es", name="es")
                for half in range(2):
                    pq = psum.tile([P, 2 * N], f32, tag="pqk", name="pq")
                    for j in range(2):
                        ki = 2 * half + j
                        nc.tensor.matmul(
                            pq[:, j * N:(j + 1) * N],
                            kT[b][rs, mi, ki * P:(ki + 1) * P],
                            qT[b][rs, mi, :],
                            start=True, stop=True)
                    nc.scalar.activation(
                        es[:, half * 2 * N:(half + 1) * 2 * N], pq, Exp)
                (mul_engine or nc.vector).tensor_mul(es, es, ETflat)
                es_t[(b, h)] = es

            def emit_pv_head(b, h):
                es = es_t.pop((b, h))
                po = psum.tile([P, N], f32, tag="po", name="po")
                for ki in range(NQC):
                    # [ones|v] -> Z (replicated) rows 0:64, O rows 64:128
                    nc.tensor.matmul(po, vsb[b][:, ki, h, :],
                                     es[:, ki * N:(ki + 1) * N],
                                     start=(ki == 0), stop=(ki == NQC - 1))
                # reciprocal straight from PSUM at partition offset 0 (the
                # fast-recip uop only works at offset 0 on HW), then scale O
                # into its OT slot: mixed-space inputs may use different base
                # partitions, and the output offset is free.
                rr = zp.tile([P, N], f32, tag="rr", name="rr")
                nc.vector.reciprocal_approx_fast(rr[0:HD, :], po[0:HD, :])
                orow = slice((h % 2) * HD, (h % 2) * HD + HD)
                nc.vector.tensor_mul(OT[b][orow, h // 2, :], po[HD:P, :],
                                     rr[0:HD, :])

            def emit_y(b, ni):
                ysb = op_.tile([P, H], f32, tag="ysb", name="ysb", bufs=3)
                for half in range(2):
                    cols = slice(0, N) if half == 0 else slice(N, H)
                    width = cols.stop - cols.start
                    py = psum.tile([P, N], f32, tag="pp", name="py")
                    for jc in range(KC):
                        nc.tensor.matmul(
                            py[:, 0:width],
                            OT[b][:, jc, ni * P:(ni + 1) * P],
                            woT[:, jc, cols],
                            start=(jc == 0), stop=False)
                    nc.tensor.matmul(py[:, 0:width], ones_r1,
                                     bo2_sb[:, cols], start=False, stop=True)
                    if half == 0:
                        nc.scalar.copy(ysb[:, cols], py[:, 0:width])
                    else:
                        nc.vector.tensor_copy(ysb[:, cols], py[:, 0:width])
                nc.sync.dma_start(
                    out=y_d[b, ni * P:(ni + 1) * P, :], in_=ysb)

            # ---------------- emission schedule ----------------
            wkT = weights.tile([P, KC, H], bf16)
            nc.sync.dma_start(out=wkT, in_=wk_d)
            bsum0 = emit_bias_dma(0)
            wvT = weights.tile([P, KC, H], bf16)
            nc.sync.dma_start(out=wvT, in_=wv_d)
            for t in vsb:
                nc.gpsimd.memset(t[:, :, :, 0:HD], 1.0)
            xb_t1 = io.tile([P, KC, N], bf16, tag="xb", name="xb1")
            nc.sync.dma_start(out=xb_t1, in_=xb_d[1])
            xbt.append(xb_t1)
            woT = weights.tile([P, KC, H], bf16)
            nc.sync.dma_start(out=woT, in_=wo_d)
            bo2_sb = consts.tile([1, H], bf16)
            nc.sync.dma_start(out=bo2_sb, in_=bo_d[None, :])
            bsum1 = emit_bias_dma(1)
            for b in range(PB):
                qT[b] = qkp.tile([P, KC, N], bf16, tag="qT", name="qT")
                kT[b] = qkp.tile([P, KC, N], bf16, tag="kT", name="kT")
                OT[b] = op_.tile([P, KC, N], bf16, tag="OT", name="OT")

            for mi in range(KC):
                emit_projqk(0, mi, "q")
            for mi in range(KC):
                emit_projqk(0, mi, "k")
            for ni in range(NQC):
                emit_projv(0, ni)
            ET0 = emit_bias_exp(bsum0, nc.vector)
            ETs = {0: ET0}

            # attention(0) interleaved with projections(1)
            b1_chunks = ([("qk", mi) for mi in range(KC)]
                         + [("v", ni) for ni in range(NQC)])
            ci = 0
            for h in range(NH):
                emit_qk_head(0, h, ETs[0])
                if h == 2:
                    ETs[1] = emit_bias_exp(bsum1, nc.gpsimd)
                if h >= 2:
                    emit_pv_head(0, h - 2)
                if ci < len(b1_chunks):
                    kind, idx = b1_chunks[ci]
                    ci += 1
                    (emit_projqk if kind == "qk" else emit_projv)(1, idx)
            while ci < len(b1_chunks):
                kind, idx = b1_chunks[ci]
                ci += 1
                (emit_projqk if kind == "qk" else emit_projv)(1, idx)
            emit_pv_head(0, NH - 2)
            emit_pv_head(0, NH - 1)

            # attention(1) interleaved with output proj(0)
            for h in range(NH):
                emit_qk_head(1, h, ETs[1])
                if h >= 2:
                    emit_pv_head(1, h - 2)
                if h % 3 == 1:
                    emit_y(0, h // 3)
            emit_pv_head(1, NH - 2)
            emit_pv_head(1, NH - 1)
            for ni in range(NQC):
                emit_y(1, ni)

    nc.compile()
    return nc


def _pack_inputs(x, sp, ed, Wq, Wk, Wv, bv, Wo, bo):
    """Host-side layout/dtype marshalling (pure data movement + weight
    preprocessing; all activation arithmetic happens on-device)."""
    x = np.asarray(x, np.float32)
    sp = np.asarray(sp, np.float32)
    ed = np.asarray(ed, np.float32)
    Wq = np.asarray(Wq, np.float32)
    Wk = np.asarray(Wk, np.float32)
    Wv = np.asarray(Wv, np.float32)
    Wo = np.asarray(Wo, np.float32)
    bv = np.asarray(bv, np.float32)
    bo = np.asarray(bo, np.float32)

    xT = np.ascontiguousarray(x.transpose(0, 2, 1))          # [B, H, N]
    xb = np.ascontiguousarray(
        xT.reshape(B, KC, P, N).transpose(0, 2, 1, 3)).astype(BFNP)
    spT = np.ascontiguousarray(
        sp.transpose(0, 2, 1).reshape(B, NQC, P, N).transpose(0, 2, 1, 3))
    edT = np.ascontiguousarray(
        ed.transpose(0, 2, 1).reshape(B, NQC, P, N).transpose(0, 2, 1, 3))

    def packb(W, mul=1.0):
        WT = np.ascontiguousarray(W.T * mul)
        return np.ascontiguousarray(
            WT.reshape(KC, P, H).transpose(1, 0, 2)).astype(BFNP)

    shared = {
        "wqT": packb(Wq, SCALE), "wkT": packb(Wk),
        "wvT": packb(Wv), "woT": packb(Wo),
        "bo2": (bo + bv @ Wo.T).astype(BFNP),
    }
    return xb, spT, edT, shared


def kernel(x, spatial_encoding, edge_encoding, Wq, Wk, Wv, bv, Wo, bo):
    global _COMPILED
    from concourse.bass_utils import run_bass_kernel_spmd

    if _COMPILED is None:
        _COMPILED = _build()
    nc = _COMPILED

    xb, spT, edT, shared = _pack_inputs(
        x, spatial_encoding, edge_encoding, Wq, Wk, Wv, bv, Wo, bo)

    in_maps = []
    for c in range(NCORES):
        sl = slice(c * PB, (c + 1) * PB)
        in_maps.append({"xb": xb[sl],
                        "spT": spT[sl], "edT": edT[sl], **shared})

    res = run_bass_kernel_spmd(nc, in_maps, list(range(NCORES)))
    return np.concatenate([res.results[c]["y"] for c in range(NCORES)], axis=0)


# revision 31
# speedup vs baseline: 1.0145x; 1.0145x over previous
"""Graphormer multi-head attention on 8 Trainium2 NeuronCores.

Problem (hardcoded): B=16, N=512, HIDDEN=768, 12 heads x 64.
  q = x @ Wq.T ; k = x @ Wk.T ; v = x @ Wv.T + bv
  scores = (q.k / sqrt(768)) + (spatial + edge)[:, None]
  out = softmax(scores) @ v ; y = out @ Wo.T + bo
Sharding: data-parallel over batch, 2 batches per core on 8 cores.

Per-core kernel strategy:
  - All layout work happens on the HOST: x/spatial/edge are pre-transposed
    and packed; weights are pre-transposed and cast to bf16.  bv is
    folded into bo' = bo + Wo@bv on the host (valid because softmax rows
    sum to 1).  No PE transposes.
  - All matmuls bf16 with fp32 PSUM accumulation; the 1/sqrt(768)
    softmax scale is folded into Wq on the host.
  - Attention in the S^T layout: S.T[nk, nq] = kT.T @ qT per head.
    exp(S^T * s) on ScalarE -> bf16, then one DVE multiply with
    E^T = exp((spatial+edge)^T) shared across heads.
  - PV per head: lhsT = [v_head | ones] (ones live in dedicated slots of
    the v tile), so one matmul chain yields both O^T (64 rows) and the
    softmax denominator replicated across the other 64 rows -- no
    separate row-sum matmuls.  A tiny stride-0-partition DMA broadcasts
    the denominator row into SBUF on the partitions where O^T lives,
    keeping the reciprocal+normalize DVE ops partition-aligned.
  - y = O @ Wo.T + bo' with lhsT = O^T; bo' enters as a K=1 matmul and
    the result is DMA'd to DRAM straight out of PSUM.
  - Emission interleaves batch b+1's projections into batch b's
    attention loop so the PE never starves.
"""

import numpy as np
import ml_dtypes

B, N, H = 16, 512, 768
NH, HD = 12, 64
NCORES = 8
PB = B // NCORES  # batches per core
P = 128
KC = H // P   # 6 hidden chunks of 128
NQC = N // P  # 4 sequence chunks of 128
SCALE = float(H) ** -0.5        # folded into Wq on the host

BFNP = ml_dtypes.bfloat16

_COMPILED = None

# debug toggles (set before _build for HW-vs-sim bisection)
DBG_NO_PBCAST = False    # replace partition_broadcast with memset(1.0)
DBG_NO_ACCUM = False     # replace DMA-accum bias sum with DVE add
DBG_NO_GMEMSET = False   # vsb ones via DVE memset instead of gpsimd
DBG_SEQUENTIAL = False   # no cross-batch interleaving in emission order
DBG_DUMP = False         # DMA b1 intermediates to DRAM debug outputs


def _build():
    import concourse.bacc as bacc
    import concourse.tile as tile
    import concourse.mybir as mybir

    f32 = mybir.dt.float32
    bf16 = mybir.dt.bfloat16
    Exp = mybir.ActivationFunctionType.Exp
    ADD = mybir.AluOpType.add

    nc = bacc.Bacc("TRN2", target_bir_lowering=False, debug=False,
                   enable_asserts=False, num_devices=NCORES)

    xb_d = nc.dram_tensor("xb", [PB, P, KC, N], bf16, kind="ExternalInput").ap()
    sp_d = nc.dram_tensor("spT", [PB, P, NQC, N], f32, kind="ExternalInput").ap()
    ed_d = nc.dram_tensor("edT", [PB, P, NQC, N], f32, kind="ExternalInput").ap()
    wq_d = nc.dram_tensor("wqT", [P, KC, H], bf16, kind="ExternalInput").ap()
    wk_d = nc.dram_tensor("wkT", [P, KC, H], bf16, kind="ExternalInput").ap()
    wv_d = nc.dram_tensor("wvT", [P, KC, H], bf16, kind="ExternalInput").ap()
    wo_d = nc.dram_tensor("woT", [P, KC, H], bf16, kind="ExternalInput").ap()
    bo_d = nc.dram_tensor("bo2", [H], bf16, kind="ExternalInput").ap()
    y_d = nc.dram_tensor("y", [PB, N, H], f32, kind="ExternalOutput").ap()

    with tile.TileContext(nc) as tc:
        with (
            tc.tile_pool(name="consts", bufs=1) as consts,
            tc.tile_pool(name="weights", bufs=1) as weights,
            tc.tile_pool(name="io", bufs=2) as io,
            tc.tile_pool(name="biasp", bufs=2) as biasp,
            tc.tile_pool(name="qk", bufs=2) as qkp,
            tc.tile_pool(name="vp", bufs=1) as vp,
            tc.tile_pool(name="soft", bufs=5) as soft,
            tc.tile_pool(name="zp", bufs=2) as zp,
            tc.tile_pool(name="op", bufs=2) as op_,
            tc.tile_pool(name="psum", bufs=2, space="PSUM") as psum,
        ):
            ones_r1 = consts.tile([1, P], bf16)
            nc.vector.memset(ones_r1, 1.0)

            # v tiles: each head owns a contiguous 128-wide slot holding
            # [ones | v_h]; the ones columns make the PV matmul emit the
            # softmax denominator, replicated across PSUM partitions 0:64,
            # for free (O lands on 64:128).
            vsb = [vp.tile([P, NQC, NH, P], bf16, name=f"vsb{i}")
                   for i in range(PB)]

            # DMA issue order is the PE warm-up critical path: the q
            # projection needs wqT + xb(0) first, then the bias sum (for the
            # first attention exp), then wkT; everything else can trail.
            wqT = weights.tile([P, KC, H], bf16)
            xbt = []
            xb_t0 = io.tile([P, KC, N], bf16, tag="xb", name="xb0")
            for kc in range(KC):
                nc.sync.dma_start(out=wqT[:, kc, :], in_=wq_d[:, kc, :])
                nc.sync.dma_start(out=xb_t0[:, kc, :], in_=xb_d[0, :, kc, :])
            xbt.append(xb_t0)

            def emit_bias_dma(b):
                bsum = biasp.tile([P, NQC, N], f32, tag="bsum", name="bsum")
                esb = biasp.tile([P, NQC, N], f32, tag="esb", name="esb")
                nc.sync.dma_start(out=bsum, in_=sp_d[b])
                nc.sync.dma_start(out=esb, in_=ed_d[b])
                return bsum, esb

            def emit_bias_exp(bs, engine):
                bsum, esb = bs
                engine.tensor_add(bsum, bsum, esb)
                ET = biasp.tile([P, NQC, N], bf16, tag="ET", name="ET")
                nc.scalar.activation(ET, bsum, Exp)
                return ET.rearrange("p c n -> p (c n)")

            qT, kT, OT = {}, {}, {}

            def emit_projqk(b, mi, which="qk"):
                pairs = {"q": ((wqT, qT),), "k": ((wkT, kT),),
                         "qk": ((wqT, qT), (wkT, kT))}[which]
                for wt, dst in pairs:
                    pp = psum.tile([P, N], f32, tag="pp", name="pp")
                    for kc in range(KC):
                        nc.tensor.matmul(
                            pp, wt[:, kc, mi * P:(mi + 1) * P],
                            xbt[b][:, kc, :],
                            start=(kc == 0), stop=(kc == KC - 1))
                    nc.scalar.copy(dst[b][:, mi, :], pp)

            def emit_projv(b, ni):
                for half in range(2):
                    cols = slice(0, N) if half == 0 else slice(N, H)
                    width = cols.stop - cols.start
                    nh = width // HD       # heads covered by this psum tile
                    h0 = half * 8          # first head
                    pv = psum.tile([P, N], f32, tag="pp", name="pv")
                    for kc in range(KC):
                        nc.tensor.matmul(
                            pv[:, 0:width],
                            xbt[b][:, kc, ni * P:(ni + 1) * P],
                            wvT[:, kc, cols],
                            start=(kc == 0), stop=(kc == KC - 1))
                    pvh = pv[:, 0:width].rearrange("p (h d) -> p h d", d=HD)
                    nc.vector.tensor_copy(
                        vsb[b][:, ni, h0:h0 + nh, HD:P], pvh)

            es_t = {}

            def emit_qk_head(b, h, ETflat, mul_engine=None):
                mi, r0 = h // 2, (h % 2) * HD
                rs = slice(r0, r0 + HD)
                es = soft.tile([P, 2 * N * 2], bf16, tag="es", name="es")
                for half in range(2):
                    pq = psum.tile([P, 2 * N], f32, tag="pqk", name="pq")
                    for j in range(2):
                        ki = 2 * half + j
                        nc.tensor.matmul(
                            pq[:, j * N:(j + 1) * N],
                            kT[b][rs, mi, ki * P:(ki + 1) * P],
                            qT[b][rs, mi, :],
                            start=True, stop=True)
                    nc.scalar.activation(
                        es[:, half * 2 * N:(half + 1) * 2 * N], pq, Exp)
                (mul_engine or nc.vector).tensor_mul(es, es, ETflat)
                es_t[(b, h)] = es

            def emit_pv_head(b, h):
                es = es_t.pop((b, h))
                po = psum.tile([P, N], f32, tag="po", name="po")
                for ki in range(NQC):
                    # [ones|v] -> Z (replicated) rows 0:64, O rows 64:128
                    nc.tensor.matmul(po, vsb[b][:, ki, h, :],
                                     es[:, ki * N:(ki + 1) * N],
                                     start=(ki == 0), stop=(ki == NQC - 1))
                # reciprocal straight from PSUM at partition offset 0 (the
                # fast-recip uop only works at offset 0 on HW), then scale O
                # into its OT slot: mixed-space inputs may use different base
                # partitions, and the output offset is free.
                rr = zp.tile([P, N], f32, tag="rr", name="rr")
                nc.vector.reciprocal_approx_fast(rr[0:HD, :], po[0:HD, :])
                orow = slice((h % 2) * HD, (h % 2) * HD + HD)
                nc.vector.tensor_mul(OT[b][orow, h // 2, :], po[HD:P, :],
                                     rr[0:HD, :])

            def emit_y(b, ni):
                ysb = op_.tile([P, H], f32, tag="ysb", name="ysb", bufs=3)
                for half in range(2):
                    cols = slice(0, N) if half == 0 else slice(N, H)
                    width = cols.stop - cols.start
                    py = psum.tile([P, N], f32, tag="pp", name="py")
                    for jc in range(KC):
                        nc.tensor.matmul(
                            py[:, 0:width],
                            OT[b][:, jc, ni * P:(ni + 1) * P],
                            woT[:, jc, cols],
                            start=(jc == 0), stop=False)
                    nc.tensor.matmul(py[:, 0:width], ones_r1,
                                     bo2_sb[:, cols], start=False, stop=True)
                    if half == 0:
                        nc.scalar.copy(ysb[:, cols], py[:, 0:width])
                    else:
                        nc.vector.tensor_copy(ysb[:, cols], py[:, 0:width])
                nc.sync.dma_start(
                    out=y_d[b, ni * P:(ni + 1) * P, :], in_=ysb)

            # ---------------- emission schedule ----------------
            wkT = weights.tile([P, KC, H], bf16)
            nc.sync.dma_start(out=wkT, in_=wk_d)
            bsum0 = emit_bias_dma(0)
            wvT = weights.tile([P, KC, H], bf16)
            nc.sync.dma_start(out=wvT, in_=wv_d)
            for t in vsb:
                nc.gpsimd.memset(t[:, :, :, 0:HD], 1.0)
            xb_t1 = io.tile([P, KC, N], bf16, tag="xb", name="xb1")
            nc.sync.dma_start(out=xb_t1, in_=xb_d[1])
            xbt.append(xb_t1)
            woT = weights.tile([P, KC, H], bf16)
            nc.sync.dma_start(out=woT, in_=wo_d)
            bo2_sb = consts.tile([1, H], bf16)
            nc.sync.dma_start(out=bo2_sb, in_=bo_d[None, :])
            bsum1 = emit_bias_dma(1)
            for b in range(PB):
                qT[b] = qkp.tile([P, KC, N], bf16, tag="qT", name="qT")
                kT[b] = qkp.tile([P, KC, N], bf16, tag="kT", name="kT")
                OT[b] = op_.tile([P, KC, N], bf16, tag="OT", name="OT")

            for mi in range(KC):
                emit_projqk(0, mi, "q")
            for mi in range(KC):
                emit_projqk(0, mi, "k")
            ET0 = emit_bias_exp(bsum0, nc.vector)
            ETs = {0: ET0}
            # pre-fill the attention pipeline: heads 0/1's scores + exp run
            # while the PE does the v projection.
            emit_qk_head(0, 0, ET0)
            emit_qk_head(0, 1, ET0)
            for ni in range(NQC):
                emit_projv(0, ni)

            # attention(0) interleaved with projections(1)
            b1_chunks = ([("qk", mi) for mi in range(KC)]
                         + [("v", ni) for ni in range(NQC)])
            ci = 0
            for h in range(2, NH):
                emit_qk_head(0, h, ETs[0])
                if h == 2:
                    ETs[1] = emit_bias_exp(bsum1, nc.gpsimd)
                emit_pv_head(0, h - 2)
                if ci < len(b1_chunks):
                    kind, idx = b1_chunks[ci]
                    ci += 1
                    (emit_projqk if kind == "qk" else emit_projv)(1, idx)
            while ci < len(b1_chunks):
                kind, idx = b1_chunks[ci]
                ci += 1
                (emit_projqk if kind == "qk" else emit_projv)(1, idx)
            emit_qk_head(1, 0, ETs[1])
            emit_pv_head(0, NH - 2)
            emit_qk_head(1, 1, ETs[1])
            emit_pv_head(0, NH - 1)

            # attention(1) interleaved with output proj(0)
            yi = 0
            for h in range(2, NH):
                emit_qk_head(1, h, ETs[1])
                emit_pv_head(1, h - 2)
                if h % 3 == 1:
                    emit_y(0, yi)
                    yi += 1
            emit_y(0, 3)
            emit_pv_head(1, NH - 2)
            emit_pv_head(1, NH - 1)
            for ni in range(NQC):
                emit_y(1, ni)

    nc.compile()
    return nc


def _pack_inputs(x, sp, ed, Wq, Wk, Wv, bv, Wo, bo):
    """Host-side layout/dtype marshalling (pure data movement + weight
    preprocessing; all activation arithmetic happens on-device)."""
    x = np.asarray(x, np.float32)
    sp = np.asarray(sp, np.float32)
    ed = np.asarray(ed, np.float32)
    Wq = np.asarray(Wq, np.float32)
    Wk = np.asarray(Wk, np.float32)
    Wv = np.asarray(Wv, np.float32)
    Wo = np.asarray(Wo, np.float32)
    bv = np.asarray(bv, np.float32)
    bo = np.asarray(bo, np.float32)

    xT = np.ascontiguousarray(x.transpose(0, 2, 1))          # [B, H, N]
    xb = np.ascontiguousarray(
        xT.reshape(B, KC, P, N).transpose(0, 2, 1, 3)).astype(BFNP)
    spT = np.ascontiguousarray(
        sp.transpose(0, 2, 1).reshape(B, NQC, P, N).transpose(0, 2, 1, 3))
    edT = np.ascontiguousarray(
        ed.transpose(0, 2, 1).reshape(B, NQC, P, N).transpose(0, 2, 1, 3))

    def packb(W, mul=1.0):
        WT = np.ascontiguousarray(W.T * mul)
        return np.ascontiguousarray(
            WT.reshape(KC, P, H).transpose(1, 0, 2)).astype(BFNP)

    shared = {
        "wqT": packb(Wq, SCALE), "wkT": packb(Wk),
        "wvT": packb(Wv), "woT": packb(Wo),
        "bo2": (bo + bv @ Wo.T).astype(BFNP),
    }
    return xb, spT, edT, shared


def kernel(x, spatial_encoding, edge_encoding, Wq, Wk, Wv, bv, Wo, bo):
    global _COMPILED
    from concourse.bass_utils import run_bass_kernel_spmd

    if _COMPILED is None:
        _COMPILED = _build()
    nc = _COMPILED

    xb, spT, edT, shared = _pack_inputs(
        x, spatial_encoding, edge_encoding, Wq, Wk, Wv, bv, Wo, bo)

    in_maps = []
    for c in range(NCORES):
        sl = slice(c * PB, (c + 1) * PB)
        in_maps.append({"xb": xb[sl],
                        "spT": spT[sl], "edT": edT[sl], **shared})

    res = run_bass_kernel_spmd(nc, in_maps, list(range(NCORES)))
    return np.concatenate([res.results[c]["y"] for c in range(NCORES)], axis=0)


# revision 33
# speedup vs baseline: 1.0166x; 1.0021x over previous
"""Graphormer multi-head attention on 8 Trainium2 NeuronCores.

Problem (hardcoded): B=16, N=512, HIDDEN=768, 12 heads x 64.
  q = x @ Wq.T ; k = x @ Wk.T ; v = x @ Wv.T + bv
  scores = (q.k / sqrt(768)) + (spatial + edge)[:, None]
  out = softmax(scores) @ v ; y = out @ Wo.T + bo
Sharding: data-parallel over batch, 2 batches per core on 8 cores.

Per-core kernel strategy:
  - All layout work happens on the HOST: x/spatial/edge are pre-transposed
    and packed; weights are pre-transposed and cast to bf16.  bv is
    folded into bo' = bo + Wo@bv on the host (valid because softmax rows
    sum to 1).  No PE transposes.
  - All matmuls bf16 with fp32 PSUM accumulation; the 1/sqrt(768)
    softmax scale is folded into Wq on the host.
  - Attention in the S^T layout: S.T[nk, nq] = kT.T @ qT per head.
    exp(S^T * s) on ScalarE -> bf16, then one DVE multiply with
    E^T = exp((spatial+edge)^T) shared across heads.
  - PV per head: lhsT = [v_head | ones] (ones live in dedicated slots of
    the v tile), so one matmul chain yields both O^T (64 rows) and the
    softmax denominator replicated across the other 64 rows -- no
    separate row-sum matmuls.  A tiny stride-0-partition DMA broadcasts
    the denominator row into SBUF on the partitions where O^T lives,
    keeping the reciprocal+normalize DVE ops partition-aligned.
  - y = O @ Wo.T + bo' with lhsT = O^T; bo' enters as a K=1 matmul and
    the result is DMA'd to DRAM straight out of PSUM.
  - Emission interleaves batch b+1's projections into batch b's
    attention loop so the PE never starves.
"""

import numpy as np
import ml_dtypes

B, N, H = 16, 512, 768
NH, HD = 12, 64
NCORES = 8
PB = B // NCORES  # batches per core
P = 128
KC = H // P   # 6 hidden chunks of 128
NQC = N // P  # 4 sequence chunks of 128
SCALE = float(H) ** -0.5        # folded into Wq on the host

BFNP = ml_dtypes.bfloat16

_COMPILED = None

# debug toggles (set before _build for HW-vs-sim bisection)
DBG_NO_PBCAST = False    # replace partition_broadcast with memset(1.0)
DBG_NO_ACCUM = False     # replace DMA-accum bias sum with DVE add
DBG_NO_GMEMSET = False   # vsb ones via DVE memset instead of gpsimd
DBG_SEQUENTIAL = False   # no cross-batch interleaving in emission order
DBG_DUMP = False         # DMA b1 intermediates to DRAM debug outputs


def _build():
    import concourse.bacc as bacc
    import concourse.tile as tile
    import concourse.mybir as mybir

    f32 = mybir.dt.float32
    bf16 = mybir.dt.bfloat16
    Exp = mybir.ActivationFunctionType.Exp
    ADD = mybir.AluOpType.add

    nc = bacc.Bacc("TRN2", target_bir_lowering=False, debug=False,
                   enable_asserts=False, num_devices=NCORES)

    xb_d = nc.dram_tensor("xb", [PB, P, KC, N], bf16, kind="ExternalInput").ap()
    sp_d = nc.dram_tensor("spT", [PB, P, NQC, N], f32, kind="ExternalInput").ap()
    ed_d = nc.dram_tensor("edT", [PB, P, NQC, N], f32, kind="ExternalInput").ap()
    wq_d = nc.dram_tensor("wqT", [P, KC, H], bf16, kind="ExternalInput").ap()
    wk_d = nc.dram_tensor("wkT", [P, KC, H], bf16, kind="ExternalInput").ap()
    wv_d = nc.dram_tensor("wvT", [P, KC, H], bf16, kind="ExternalInput").ap()
    wo_d = nc.dram_tensor("woT", [P, KC, H], bf16, kind="ExternalInput").ap()
    bo_d = nc.dram_tensor("bo2", [H], bf16, kind="ExternalInput").ap()
    y_d = nc.dram_tensor("y", [PB, N, H], f32, kind="ExternalOutput").ap()

    with tile.TileContext(nc) as tc:
        with (
            tc.tile_pool(name="consts", bufs=1) as consts,
            tc.tile_pool(name="weights", bufs=1) as weights,
            tc.tile_pool(name="io", bufs=2) as io,
            tc.tile_pool(name="biasp", bufs=2) as biasp,
            tc.tile_pool(name="qk", bufs=2) as qkp,
            tc.tile_pool(name="vp", bufs=1) as vp,
            tc.tile_pool(name="soft", bufs=5) as soft,
            tc.tile_pool(name="zp", bufs=2) as zp,
            tc.tile_pool(name="op", bufs=2) as op_,
            tc.tile_pool(name="psum", bufs=2, space="PSUM") as psum,
        ):
            ones_r1 = consts.tile([1, P], bf16)
            nc.vector.memset(ones_r1, 1.0)

            # v tiles: each head owns a contiguous 128-wide slot holding
            # [ones | v_h]; the ones columns make the PV matmul emit the
            # softmax denominator, replicated across PSUM partitions 0:64,
            # for free (O lands on 64:128).
            vsb = [vp.tile([P, NQC, NH, P], bf16, name=f"vsb{i}")
                   for i in range(PB)]

            # DMA issue order is the PE warm-up critical path: the q
            # projection needs wqT + xb(0) first, then the bias sum (for the
            # first attention exp), then wkT; everything else can trail.
            wqT = weights.tile([P, KC, H], bf16)
            xbt = []
            xb_t0 = io.tile([P, KC, N], bf16, tag="xb", name="xb0")
            for kc in range(KC):
                nc.sync.dma_start(out=wqT[:, kc, :], in_=wq_d[:, kc, :])
                nc.scalar.dma_start(out=xb_t0[:, kc, :], in_=xb_d[0, :, kc, :])
            xbt.append(xb_t0)

            def emit_bias_dma(b):
                bsum = biasp.tile([P, NQC, N], f32, tag="bsum", name="bsum")
                esb = biasp.tile([P, NQC, N], f32, tag="esb", name="esb")
                nc.sync.dma_start(out=bsum, in_=sp_d[b])
                nc.sync.dma_start(out=esb, in_=ed_d[b])
                return bsum, esb

            def emit_bias_exp(bs, engine):
                bsum, esb = bs
                engine.tensor_add(bsum, bsum, esb)
                ET = biasp.tile([P, NQC, N], bf16, tag="ET", name="ET")
                nc.scalar.activation(ET, bsum, Exp)
                return ET.rearrange("p c n -> p (c n)")

            qT, kT, OT = {}, {}, {}

            def emit_projqk(b, mi, which="qk"):
                pairs = {"q": ((wqT, qT),), "k": ((wkT, kT),),
                         "qk": ((wqT, qT), (wkT, kT))}[which]
                for wt, dst in pairs:
                    pp = psum.tile([P, N], f32, tag="pp", name="pp")
                    for kc in range(KC):
                        nc.tensor.matmul(
                            pp, wt[:, kc, mi * P:(mi + 1) * P],
                            xbt[b][:, kc, :],
                            start=(kc == 0), stop=(kc == KC - 1))
                    nc.scalar.copy(dst[b][:, mi, :], pp)

            def emit_projv(b, ni):
                for half in range(2):
                    cols = slice(0, N) if half == 0 else slice(N, H)
                    width = cols.stop - cols.start
                    nh = width // HD       # heads covered by this psum tile
                    h0 = half * 8          # first head
                    pv = psum.tile([P, N], f32, tag="pp", name="pv")
                    for kc in range(KC):
                        nc.tensor.matmul(
                            pv[:, 0:width],
                            xbt[b][:, kc, ni * P:(ni + 1) * P],
                            wvT[:, kc, cols],
                            start=(kc == 0), stop=(kc == KC - 1))
                    pvh = pv[:, 0:width].rearrange("p (h d) -> p h d", d=HD)
                    nc.vector.tensor_copy(
                        vsb[b][:, ni, h0:h0 + nh, HD:P], pvh)

            es_t = {}

            def emit_qk_head(b, h, ETflat, mul_engine=None):
                mi, r0 = h // 2, (h % 2) * HD
                rs = slice(r0, r0 + HD)
                es = soft.tile([P, 2 * N * 2], bf16, tag="es", name="es")
                for half in range(2):
                    pq = psum.tile([P, 2 * N], f32, tag="pqk", name="pq")
                    for j in range(2):
                        ki = 2 * half + j
                        nc.tensor.matmul(
                            pq[:, j * N:(j + 1) * N],
                            kT[b][rs, mi, ki * P:(ki + 1) * P],
                            qT[b][rs, mi, :],
                            start=True, stop=True)
                    nc.scalar.activation(
                        es[:, half * 2 * N:(half + 1) * 2 * N], pq, Exp)
                (mul_engine or nc.vector).tensor_mul(es, es, ETflat)
                es_t[(b, h)] = es

            def emit_pv_head(b, h):
                es = es_t.pop((b, h))
                po = psum.tile([P, N], f32, tag="po", name="po")
                for ki in range(NQC):
                    # [ones|v] -> Z (replicated) rows 0:64, O rows 64:128
                    nc.tensor.matmul(po, vsb[b][:, ki, h, :],
                                     es[:, ki * N:(ki + 1) * N],
                                     start=(ki == 0), stop=(ki == NQC - 1))
                # reciprocal straight from PSUM at partition offset 0 (the
                # fast-recip uop only works at offset 0 on HW), then scale O
                # into its OT slot: mixed-space inputs may use different base
                # partitions, and the output offset is free.
                rr = zp.tile([P, N], f32, tag="rr", name="rr")
                nc.vector.reciprocal_approx_fast(rr[0:HD, :], po[0:HD, :])
                orow = slice((h % 2) * HD, (h % 2) * HD + HD)
                nc.vector.tensor_mul(OT[b][orow, h // 2, :], po[HD:P, :],
                                     rr[0:HD, :])

            def emit_y(b, ni):
                ysb = op_.tile([P, H], f32, tag="ysb", name="ysb", bufs=3)
                for half in range(2):
                    cols = slice(0, N) if half == 0 else slice(N, H)
                    width = cols.stop - cols.start
                    py = psum.tile([P, N], f32, tag="pp", name="py")
                    for jc in range(KC):
                        nc.tensor.matmul(
                            py[:, 0:width],
                            OT[b][:, jc, ni * P:(ni + 1) * P],
                            woT[:, jc, cols],
                            start=(jc == 0), stop=False)
                    nc.tensor.matmul(py[:, 0:width], ones_r1,
                                     bo2_sb[:, cols], start=False, stop=True)
                    if half == 0:
                        nc.scalar.copy(ysb[:, cols], py[:, 0:width])
                    else:
                        nc.vector.tensor_copy(ysb[:, cols], py[:, 0:width])
                nc.sync.dma_start(
                    out=y_d[b, ni * P:(ni + 1) * P, :], in_=ysb)

            # ---------------- emission schedule ----------------
            wkT = weights.tile([P, KC, H], bf16)
            nc.sync.dma_start(out=wkT, in_=wk_d)
            bsum0 = emit_bias_dma(0)
            wvT = weights.tile([P, KC, H], bf16)
            nc.sync.dma_start(out=wvT, in_=wv_d)
            for t in vsb:
                nc.gpsimd.memset(t[:, :, :, 0:HD], 1.0)
            xb_t1 = io.tile([P, KC, N], bf16, tag="xb", name="xb1")
            nc.sync.dma_start(out=xb_t1, in_=xb_d[1])
            xbt.append(xb_t1)
            woT = weights.tile([P, KC, H], bf16)
            nc.sync.dma_start(out=woT, in_=wo_d)
            bo2_sb = consts.tile([1, H], bf16)
            nc.sync.dma_start(out=bo2_sb, in_=bo_d[None, :])
            bsum1 = emit_bias_dma(1)
            for b in range(PB):
                qT[b] = qkp.tile([P, KC, N], bf16, tag="qT", name="qT")
                kT[b] = qkp.tile([P, KC, N], bf16, tag="kT", name="kT")
                OT[b] = op_.tile([P, KC, N], bf16, tag="OT", name="OT")

            for mi in range(KC):
                emit_projqk(0, mi, "q")
            ET0 = emit_bias_exp(bsum0, nc.vector)
            ETs = {0: ET0}
            for mi in range(KC):
                emit_projqk(0, mi, "k")
            # pre-fill the attention pipeline: heads 0/1's scores + exp run
            # while the PE does the v projection.
            emit_qk_head(0, 0, ET0)
            emit_qk_head(0, 1, ET0)
            for ni in range(NQC):
                emit_projv(0, ni)

            # attention(0) interleaved with projections(1)
            b1_chunks = ([("qk", mi) for mi in range(KC)]
                         + [("v", ni) for ni in range(NQC)])
            ci = 0
            for h in range(2, NH):
                emit_qk_head(0, h, ETs[0])
                if h == 2:
                    ETs[1] = emit_bias_exp(bsum1, nc.gpsimd)
                emit_pv_head(0, h - 2)
                if ci < len(b1_chunks):
                    kind, idx = b1_chunks[ci]
                    ci += 1
                    (emit_projqk if kind == "qk" else emit_projv)(1, idx)
            while ci < len(b1_chunks):
                kind, idx = b1_chunks[ci]
                ci += 1
                (emit_projqk if kind == "qk" else emit_projv)(1, idx)
            emit_qk_head(1, 0, ETs[1])
            emit_pv_head(0, NH - 2)
            emit_qk_head(1, 1, ETs[1])
            emit_pv_head(0, NH - 1)

            # attention(1) interleaved with output proj(0)
            yi = 0
            for h in range(2, NH):
                emit_qk_head(1, h, ETs[1])
                emit_pv_head(1, h - 2)
                if h % 3 == 1:
                    emit_y(0, yi)
                    yi += 1
            emit_y(0, 3)
            emit_pv_head(1, NH - 2)
            emit_pv_head(1, NH - 1)
            for ni in range(NQC):
                emit_y(1, ni)

    nc.compile()
    return nc


def _pack_inputs(x, sp, ed, Wq, Wk, Wv, bv, Wo, bo):
    """Host-side layout/dtype marshalling (pure data movement + weight
    preprocessing; all activation arithmetic happens on-device)."""
    x = np.asarray(x, np.float32)
    sp = np.asarray(sp, np.float32)
    ed = np.asarray(ed, np.float32)
    Wq = np.asarray(Wq, np.float32)
    Wk = np.asarray(Wk, np.float32)
    Wv = np.asarray(Wv, np.float32)
    Wo = np.asarray(Wo, np.float32)
    bv = np.asarray(bv, np.float32)
    bo = np.asarray(bo, np.float32)

    xT = np.ascontiguousarray(x.transpose(0, 2, 1))          # [B, H, N]
    xb = np.ascontiguousarray(
        xT.reshape(B, KC, P, N).transpose(0, 2, 1, 3)).astype(BFNP)
    spT = np.ascontiguousarray(
        sp.transpose(0, 2, 1).reshape(B, NQC, P, N).transpose(0, 2, 1, 3))
    edT = np.ascontiguousarray(
        ed.transpose(0, 2, 1).reshape(B, NQC, P, N).transpose(0, 2, 1, 3))

    def packb(W, mul=1.0):
        WT = np.ascontiguousarray(W.T * mul)
        return np.ascontiguousarray(
            WT.reshape(KC, P, H).transpose(1, 0, 2)).astype(BFNP)

    shared = {
        "wqT": packb(Wq, SCALE), "wkT": packb(Wk),
        "wvT": packb(Wv), "woT": packb(Wo),
        "bo2": (bo + bv @ Wo.T).astype(BFNP),
    }
    return xb, spT, edT, shared


def kernel(x, spatial_encoding, edge_encoding, Wq, Wk, Wv, bv, Wo, bo):
    global _COMPILED
    from concourse.bass_utils import run_bass_kernel_spmd

    if _COMPILED is None:
        _COMPILED = _build()
    nc = _COMPILED

    xb, spT, edT, shared = _pack_inputs(
        x, spatial_encoding, edge_encoding, Wq, Wk, Wv, bv, Wo, bo)

    in_maps = []
    for c in range(NCORES):
        sl = slice(c * PB, (c + 1) * PB)
        in_maps.append({"xb": xb[sl],
                        "spT": spT[sl], "edT": edT[sl], **shared})

    res = run_bass_kernel_spmd(nc, in_maps, list(range(NCORES)))
    return np.concatenate([res.results[c]["y"] for c in range(NCORES)], axis=0)
